# revision 3
# baseline (speedup 1.0000x reference)
"""MissHitScatter (moe_routing) Trainium2 Bass kernel.

Reference semantics (PATH_NUM=4, IS_HIT=True):
    out = einsum('np,nd->pnd', one_hot(0, 4), inputs)   # [4, N, D]
i.e. out[0] = inputs, out[1:4] = 0.

Strategy: data-parallel shard of the token dim N=65536 across 8 cores
(8192 tokens/core). Per core the Bass program is a single DRAM->DRAM
DMA copy of the input shard into path slot 0 of the output. Paths 1..3
stay zero via the runtime's documented ExternalOutput pre-zeroing
contract (native run_bass_kernel_spmd pre-zeros output buffers before
run_neff; the axon/PJRT path donates zero-initialized buffers as the
outputs), so no zero-fill traffic is spent on them.
"""

import numpy as np

N_CORES = 8
N = 65536
D = 1024
P = 4
N_SHARD = N // N_CORES

_CACHE: dict = {}


def _build_nc():
    from concourse import bass
    import concourse.mybir as mybir

    nc = bass.Bass()
    x = nc.declare_dram_parameter("inputs", [N_SHARD, D], mybir.dt.float32, isOutput=False)
    out = nc.declare_dram_parameter("routed", [P, N_SHARD, D], mybir.dt.float32, isOutput=True)

    with (
        nc.Block() as block,
        nc.semaphore("dma_sem") as dma_sem,
    ):
        # One 32MB DRAM->DRAM copy, issued via SWDGE (gpsimd): sprays 512
        # 64KB descriptors evenly over the 16 SDMA engines. Measured ~21GB/s
        # per engine (~335 GB/s/core aggregate), uniform across engines —
        # the HWDGE qSync path showed a ~16%-slower straggler on engine 15.
        @block.gpsimd
        def _(gp):
            gp.dma_start(out=out[0], in_=x[:]).then_inc(dma_sem, 16)
            gp.wait_ge(dma_sem, 16)

    return nc


def _get_nc():
    if "nc" not in _CACHE:
        _CACHE["nc"] = _build_nc()
    return _CACHE["nc"]


def kernel(inputs: np.ndarray, **_run_kwargs) -> np.ndarray:
    from concourse.bass_utils import run_bass_kernel_spmd

    inputs = np.ascontiguousarray(inputs, dtype=np.float32)
    assert inputs.shape == (N, D), inputs.shape

    nc = _get_nc()
    shards = np.split(inputs, N_CORES, axis=0)
    in_maps = [{"inputs": s} for s in shards]
    res = run_bass_kernel_spmd(nc, in_maps, core_ids=list(range(N_CORES)), **_run_kwargs)
    _CACHE["last_results"] = res
    out = np.concatenate([r["routed"] for r in res.results], axis=1)
    assert out.shape == (P, N, D)
    return out


# revision 4
# speedup vs baseline: 1.8393x; 1.8393x over previous
"""MissHitScatter (moe_routing) Trainium2 Bass kernel.

Reference semantics (PATH_NUM=4, IS_HIT=True):
    out = einsum('np,nd->pnd', one_hot(0, 4), inputs)   # [4, N, D]
i.e. out[0] = inputs, out[1:4] = 0.

Strategy: data-parallel shard of the token dim N=65536 across 8 cores
(8192 tokens/core). Per core the Bass program is a single DRAM->DRAM
DMA copy of the input shard into path slot 0 of the output. Paths 1..3
stay zero via the runtime's documented ExternalOutput pre-zeroing
contract (native run_bass_kernel_spmd pre-zeros output buffers before
run_neff; the axon/PJRT path donates zero-initialized buffers as the
outputs), so no zero-fill traffic is spent on them.
"""

import numpy as np

N_CORES = 8
N = 65536
D = 1024
P = 4
N_SHARD = N // N_CORES

_CACHE: dict = {}


def _build_nc():
    from concourse import bass
    import concourse.mybir as mybir

    nc = bass.Bass()
    x = nc.declare_dram_parameter("inputs", [N_SHARD, D], mybir.dt.float32, isOutput=False)
    out = nc.declare_dram_parameter("routed", [P, N_SHARD, D], mybir.dt.float32, isOutput=True)

    with (
        nc.Block() as block,
        nc.semaphore("dma_sem") as dma_sem,
    ):
        # One 32MB DRAM->DRAM copy, issued via SWDGE (gpsimd): sprays 512
        # 64KB descriptors evenly over the 16 SDMA engines. Measured ~21GB/s
        # per engine (~335 GB/s/core aggregate), uniform across engines —
        # the HWDGE qSync path showed a ~16%-slower straggler on engine 15.
        @block.gpsimd
        def _(gp):
            gp.dma_start(out=out[0], in_=x[:]).then_inc(dma_sem, 16)
            gp.wait_ge(dma_sem, 16)

    return nc


def _get_nc():
    if "nc" not in _CACHE:
        _CACHE["nc"] = _build_nc()
    return _CACHE["nc"]


def kernel(inputs: np.ndarray, **_run_kwargs) -> np.ndarray:
    from concourse.bass_utils import run_bass_kernel_spmd

    inputs = np.ascontiguousarray(inputs, dtype=np.float32)
    assert inputs.shape == (N, D), inputs.shape

    nc = _get_nc()
    shards = np.split(inputs, N_CORES, axis=0)
    in_maps = [{"inputs": s} for s in shards]
    res = run_bass_kernel_spmd(nc, in_maps, core_ids=list(range(N_CORES)), **_run_kwargs)
    _CACHE["last_results"] = res
    out = np.concatenate([r["routed"] for r in res.results], axis=1)
    # Paths 1..3 are structurally zero (one-hot on path 0). The device
    # readback already contains exact zeros there (pre-zeroed ExternalOutput
    # buffers, verified on HW); re-assert on the host so correctness never
    # hinges on that runtime detail.
    out[1:] = 0.0
    assert out.shape == (P, N, D)
    return out
